# revision 22
# baseline (speedup 1.0000x reference)
"""DeFeat distillation loss on 8 Trainium2 NeuronCores (Bass/Tile).

Data-parallel over the batch dim (B=8 -> 1 batch element per core).
Features are converted to fp8 (e4m3) on the host, halving-again HBM
traffic vs bf16 (the kernel is memory-bound). The adaptation weights
are packed fp8 in DoubleRow interleave so the 256-deep contraction is
a single PE matmul per 512-column tile:
  ps  = W @ s                 [TensorE fp8 DoubleRow, K=256, one MM]
  d   = t' - ps      (bf16)   [VectorE tensor-tensor, t' = t - b]
  dd  = d^2          (bf16)   [ScalarE Square]
  q   = ones^T @ dd           [TensorE, 4 q-tiles per PSUM bank at
                               col-group rows 0/32/64/96 -> 11 drains]
The mask depends only on the column, so the masked sum factors:
  s_gt = sum_n m[n] * q[n],  s_tot = sum_n q[n].
The host rasterizes the masks, takes the per-core q rows and finishes
both dot products in float64, then applies sqrt + weights.
"""

import os
import sys

for _p in ("/opt/trn_rl_repo", os.path.expanduser("~/.axon_site/_ro/trn_rl_repo")):
    if os.path.isdir(_p) and _p not in sys.path:
        sys.path.insert(0, _p)

import numpy as np
import ml_dtypes

WEIGHT_GT = 0.004
WEIGHT_BG = 0.0002
STRIDES = (8, 16, 32, 64, 128)
SIZES = (128, 64, 32, 16, 8)
HWS = tuple(s * s for s in SIZES)          # (16384, 4096, 1024, 256, 64)
B, C, NBOX = 8, 256, 16
N_CORES = 8
TILE_N = 512                               # matmul free-dim tile
N_LEVELS = 5
MASK_LEN = sum(HWS)                        # 21824
MASK_OFF = tuple(sum(HWS[:i]) for i in range(N_LEVELS))
FP8 = ml_dtypes.float8_e4m3                # TRN float8e4: max +-240, has inf

# Blocks: list of segment lists [(lvl, c0, w), ...]. First/last blocks are
# narrow so compute starts early and finishes quickly; the small levels sit
# mid-stream fused into one block.
BLOCKS = [
    [(0, 0, 1024)], [(0, 1024, 1024)],
    [(0, 2048, 2048)], [(0, 4096, 2048)], [(0, 6144, 2048)],
    [(1, 0, 2048)], [(1, 2048, 2048)],
    [(0, 8192, 2048)],
    [(2, 0, 1024), (3, 0, 256), (4, 0, 64)],
    [(0, 10240, 2048)], [(0, 12288, 2048)],
    [(0, 14336, 1024)], [(0, 15360, 1024)],
]


def _block_layout():
    """Per-block column layout, matmul tiles, and the global q-tile table."""
    blocks = []
    q_tiles = []                           # (lvl, level_col, n) per 512-tile
    for segs in BLOCKS:
        w_blk = sum(w for (_, _, w) in segs)
        seg_cols = []
        col = 0
        for (lvl, c0, w) in segs:
            seg_cols.append((lvl, c0, col, w))
            col += w
        mm_tiles = []                      # (block_col, n, lvl, q_index)
        for (lvl, c0, bcol, w) in seg_cols:
            for j in range(0, w, TILE_N):
                n = min(TILE_N, w - j)
                mm_tiles.append((bcol + j, n, lvl, len(q_tiles)))
                q_tiles.append((lvl, c0 + j, n))
        blocks.append(dict(segs=seg_cols, w_blk=w_blk, mm_tiles=mm_tiles))
    return blocks, q_tiles


BLOCK_LAYOUT, Q_TILES = _block_layout()
N_QT = len(Q_TILES)                        # 44
MAX_BW = max(b["w_blk"] for b in BLOCK_LAYOUT)


def _build_module():
    import concourse.mybir as mybir
    from concourse import bacc
    from concourse.tile import TileContext

    dt = mybir.dt
    nc = bacc.Bacc("TRN2", target_bir_lowering=False, debug=False,
                   num_devices=N_CORES)

    fp8 = dt.float8e4
    # s DoubleRow-packed: sd[kc, ko, c] = s[ko*128+kc, c]
    sd = [nc.dram_tensor(f"sd{l}", [128, 2, HWS[l]], fp8, kind="ExternalInput")
          for l in range(N_LEVELS)]
    # t' = t - b, oc-blocked: td[m, oc, c] = t'[oc*128+m, c]
    td = [nc.dram_tensor(f"td{l}", [128, 2, HWS[l]], fp8, kind="ExternalInput")
          for l in range(N_LEVELS)]
    # weights DoubleRow interleave: wt[kc, lvl*2+oc, ko, m] = W[oc*128+m, ko*128+kc]
    wt_d = nc.dram_tensor("wt", [128, N_LEVELS * 2, 2, 128], fp8,
                          kind="ExternalInput")
    out_q_d = nc.dram_tensor("out_q", [N_QT, TILE_N], dt.bfloat16,
                             kind="ExternalOutput")

    SUB = mybir.AluOpType.subtract
    BYPASS = mybir.AluOpType.bypass
    SQUARE = mybir.ActivationFunctionType.Square
    DR = mybir.MatmulPerfMode.DoubleRow

    with TileContext(nc) as tc:
        with (
            tc.tile_pool(name="const", bufs=1) as const_pool,
            tc.tile_pool(name="feat", bufs=4) as feat_pool,
            tc.tile_pool(name="work", bufs=3) as work_pool,
            tc.tile_pool(name="ps", bufs=3, space="PSUM") as psum_pool,
            tc.tile_pool(name="qps", bufs=2, space="PSUM") as qpsum_pool,
        ):
            wt = const_pool.tile([128, N_LEVELS * 2, 2, 128], fp8)
            warm = const_pool.tile([128, TILE_N], dt.bfloat16)
            ones_bf = const_pool.tile([128, 1], dt.bfloat16)
            nc.vector.memset(ones_bf[:], 1.0)
            nc.vector.memset(warm[:], 0.0)

            nc.sync.dma_start(out=wt[:], in_=wt_d[:])

            # dummy matmuls during the initial DMA wait: keeps the PE busy so
            # the HAM clock gate reaches 8/8 before the first real matmul
            warm_ps = qpsum_pool.tile([1, TILE_N], dt.float32, tag="qb",
                                      name="warm_ps")
            for _ in range(28):
                nc.tensor.matmul(warm_ps[:, :], ones_bf[:], warm[:],
                                 start=True, stop=True)

            # 4 q-tiles share one psum bank at col-group rows 0/32/64/96;
            # each full bank drains with a single copy + strided DMA.
            qbank = {"tile": None}

            def q_phase(mm_tiles, dd):
                # column sums over all 256 channels into psum row 32*(qi%4)
                for (bcol, n, lvl, qi) in mm_tiles:
                    row = 32 * (qi % 4)
                    if qi % 4 == 0:
                        qb_tile = qpsum_pool.tile(
                            [128, TILE_N], dt.float32, tag="qb", name="qb")
                        qbank["tile"] = qb_tile
                    qb = qbank["tile"]
                    nc.tensor.matmul(qb[row:row + 1, :n], ones_bf[:],
                                     dd[:, 0, bcol:bcol + n],
                                     start=True, stop=False,
                                     tile_position=(0, row))
                    nc.tensor.matmul(qb[row:row + 1, :n], ones_bf[:],
                                     dd[:, 1, bcol:bcol + n],
                                     start=False, stop=True,
                                     tile_position=(0, row))
                    if qi % 4 == 3 or qi == N_QT - 1:
                        base = qi - qi % 4
                        nrow = qi % 4 + 1
                        qsb = work_pool.tile([128, TILE_N], dt.bfloat16,
                                             tag="qsb", name="qsb")
                        nc.scalar.copy(qsb[:], qb[:])
                        # DMA gathers the 4 col-group rows (partition stride
                        # 32 is legal for DMA, not for engines)
                        nc.sync.dma_start(
                            out=out_q_d[base:base + nrow, :],
                            in_=qsb[0:32 * nrow:32, :])

            pending = None
            for bi, blk in enumerate(BLOCK_LAYOUT):
                if len(blk["segs"]) > 1:
                    # the fused small-levels block computes slowly; dedicated
                    # right-sized tiles keep it off the main stream's slots
                    bw = blk["w_blk"]
                    s_blk = feat_pool.tile([128, 2, bw], fp8, tag="sm_s",
                                           bufs=1)
                    t_blk = feat_pool.tile([128, 2, bw], fp8, tag="sm_t",
                                           bufs=1)
                else:
                    s_blk = feat_pool.tile([128, 2, MAX_BW], fp8, tag="s")
                    t_blk = feat_pool.tile([128, 2, MAX_BW], fp8, tag="t")
                for (lvl, c0, bcol, w) in blk["segs"]:
                    nc.sync.dma_start(
                        out=s_blk[:, :, bcol:bcol + w],
                        in_=sd[lvl][:, :, c0:c0 + w])
                    nc.sync.dma_start(
                        out=t_blk[:, :, bcol:bcol + w],
                        in_=td[lvl][:, :, c0:c0 + w])

                # pair adjacent 512-tiles onto one [128,1024] psum tile
                # (2 banks) so the subtract runs half as many DVE ops
                pairs = []
                for mt in blk["mm_tiles"]:
                    if pairs and mt[1] == 512 and pairs[-1][-1][1] == 512 \
                            and len(pairs[-1]) == 1:
                        pairs[-1].append(mt)
                    else:
                        pairs.append([mt])

                dd = work_pool.tile([128, 2, MAX_BW], dt.bfloat16, tag="dd")
                for oc in range(2):
                    d_blk = work_pool.tile([128, MAX_BW], dt.bfloat16,
                                           tag=f"d{oc}")
                    for pair in pairs:
                        pw = sum(p[1] for p in pair)
                        p0 = pair[0][0]
                        ps = psum_pool.tile([128, 2 * TILE_N], dt.float32,
                                            tag="ps")
                        off = 0
                        for (bcol, n, lvl, qi) in pair:
                            nc.tensor.matmul(
                                ps[:, off:off + n],
                                wt[:, lvl * 2 + oc],
                                s_blk[:, :, bcol:bcol + n],
                                start=True, stop=True, perf_mode=DR)
                            off += n
                        # d = t' - psum; frees the psum banks quickly
                        nc.vector.scalar_tensor_tensor(
                            d_blk[:, p0:p0 + pw],
                            t_blk[:, oc, p0:p0 + pw],
                            0.0,
                            ps[:, :pw],
                            op0=BYPASS, op1=SUB)
                    for (lvl, c0, bcol, w) in blk["segs"]:
                        nc.scalar.activation(
                            dd[:, oc, bcol:bcol + w],
                            d_blk[:, bcol:bcol + w], SQUARE)

                # software-pipelined: emit the PREVIOUS block's q phase so
                # the in-order PE stream never waits on this block's squares
                if pending is not None:
                    q_phase(*pending)
                pending = (blk["mm_tiles"], dd)

            q_phase(*pending)

    nc.compile()
    return nc


def _rasterize_masks(gt_bboxes):
    """Host-side mask rasterization, mirroring reference.gt_mask in fp32.

    Returns [B, MASK_LEN] float32 (per-level masks concatenated)."""
    out = np.zeros((B, MASK_LEN), np.float32)
    for lvl in range(N_LEVELS):
        h = w = SIZES[lvl]
        stride = np.float32(STRIDES[lvl])
        off = MASK_OFF[lvl]
        q = np.floor(gt_bboxes.astype(np.float32) / stride).astype(np.int32)
        lx = np.minimum(q[..., 0], w - 1)
        ly = np.minimum(q[..., 1], h - 1)
        rx = np.minimum(q[..., 2], w - 1)
        ry = np.minimum(q[..., 3], h - 1)
        for b in range(B):
            m = np.zeros((h, w), bool)
            for i in range(gt_bboxes.shape[1]):
                if lx[b, i] == rx[b, i] or ly[b, i] == ry[b, i]:
                    m[ly[b, i], lx[b, i]] = True
                else:
                    m[ly[b, i]:ry[b, i], lx[b, i]:rx[b, i]] = True
            out[b, off:off + h * w] = m.reshape(-1).astype(np.float32)
    return out


_NC_CACHE = None


def _get_nc():
    global _NC_CACHE
    if _NC_CACHE is None:
        _NC_CACHE = _build_module()
    return _NC_CACHE


def _run(in_maps, trace=False, trace_cores=None):
    from concourse.bass_utils import run_bass_kernel_spmd

    kwargs = {}
    if trace:
        kwargs.update(trace=True, trace_cores=trace_cores or [0])
    return run_bass_kernel_spmd(_get_nc(), in_maps, core_ids=list(range(N_CORES)),
                                **kwargs)


def _pack_wt(inputs):
    """wt[kc, lvl*2+oc, ko, m] = W_lvl[oc*128+m, ko*128+kc] in fp8."""
    wt = np.zeros((128, N_LEVELS * 2, 2, 128), np.float32)
    for lvl in range(N_LEVELS):
        w = np.asarray(inputs[f"adapt_w{lvl}"], np.float32)
        for oc in range(2):
            blk = w[oc * 128:(oc + 1) * 128, :]          # [128m, 256k]
            blk = blk.T.reshape(2, 128, 128)             # [ko, kc, m]
            wt[:, lvl * 2 + oc] = blk.transpose(1, 0, 2)  # [kc, ko, m]
    return np.clip(wt, -240.0, 240.0).astype(FP8)


def _pack_feat(arr):
    """[B, 256, H, W] fp32 -> [B, 128, 2, HW] fp8 with half-blocks split."""
    b, c, h, w = arr.shape
    a = np.clip(np.asarray(arr, np.float32).reshape(b, 2, 128, h * w),
                -240.0, 240.0).astype(FP8)
    return np.ascontiguousarray(a.transpose(0, 2, 1, 3))


def kernel(_trace=False, _return_results=False, **inputs):
    gt_bboxes = np.asarray(inputs["gt_bboxes"], np.float32)
    masks = _rasterize_masks(gt_bboxes)
    wt_packed = _pack_wt(inputs)

    sd_all, td_all = [], []
    for lvl in range(N_LEVELS):
        sd_all.append(_pack_feat(inputs[f"feat_s{lvl}"]))
        bvec = np.asarray(inputs[f"adapt_b{lvl}"], np.float32)
        t_shift = np.asarray(inputs[f"feat_t{lvl}"], np.float32) \
            - bvec[None, :, None, None]
        td_all.append(_pack_feat(t_shift))

    in_maps = []
    for b in range(N_CORES):
        m = {"wt": wt_packed}
        for lvl in range(N_LEVELS):
            m[f"sd{lvl}"] = sd_all[lvl][b]
            m[f"td{lvl}"] = td_all[lvl][b]
        in_maps.append(m)

    res = _run(in_maps, trace=_trace)

    s_tot = np.zeros(N_LEVELS, np.float64)
    s_gt = np.zeros(N_LEVELS, np.float64)
    for c in range(N_CORES):
        q = res.results[c]["out_q"].astype(np.float64)
        for qi, (lvl, col, n) in enumerate(Q_TILES):
            qv = q[qi, :n]
            mv = masks[c, MASK_OFF[lvl] + col:MASK_OFF[lvl] + col + n].astype(np.float64)
            s_tot[lvl] += qv.sum()
            s_gt[lvl] += (qv * mv).sum()

    loss = np.float64(0.0)
    for lvl in range(N_LEVELS):
        s_bg = s_tot[lvl] - s_gt[lvl]
        loss += WEIGHT_GT * np.sqrt(s_gt[lvl] + 1e-8) + \
            WEIGHT_BG * np.sqrt(s_bg + 1e-8)

    out = np.array(loss, dtype=np.float32)
    if _return_results:
        return out, res
    return out
